# revision 1
# baseline (speedup 1.0000x reference)
"""GQA causal attention (llama3-style RoPE) on 8 TRN2 NeuronCores.

Sharding: tensor-parallel over heads. Core c gets q-heads 4c..4c+3 and
kv-head c (GQA groups intact), plus the matching row-block of wo.T.
Each core computes a full [S, D] partial of the output projection;
the host sums the 8 partials (the "all-reduce" of the row-sharded wo).

Per-core pipeline (all layouts chosen so no on-device transpose of x/q/k
is ever needed):
  qkvT[col, s]  = wqkvT.T @ xT          (weights stationary, xT streaming)
  RoPE on qT/kT (de-interleaved pair layout via host-permuted weight rows)
  sT[sk, sq]    = kT.T @ qT             (K=64)
  eT            = exp(sT/8) * causal_mask
  avT[hd+1, sq] = v_aug.T @ eT          (v augmented with a ones row ->
                                         numerator and denominator in one
                                         accumulation)
  yT            = avT[0:64] * bcast(1/avT[64])
  out[sq, d]    = yT.T @ woT            (partial; host sums over cores)
"""

import sys

for _p in ("/opt/trn_rl_repo", "/root/.axon_site/_ro/trn_rl_repo"):
    if _p not in sys.path:
        sys.path.insert(0, _p)

import numpy as np
import ml_dtypes

import concourse.bass as bass
import concourse.bacc as bacc
import concourse.mybir as mybir
import concourse.tile as tile

BF16 = ml_dtypes.bfloat16

S = 2048
D = 2048
HD = 64
NH = 32
NKV = 8
NCORES = 8
QH = NH // NCORES            # 4 local q heads
QCOLS = QH * HD              # 256
KVCOLS = 2 * HD              # 128 (k and v, one kv head)
P = 128                      # partitions
NK = D // P                  # 16 contraction tiles
NSQ = S // P                 # 16 seq tiles of 128
NCH = 4                      # seq chunks of 512
CH = 512

_CACHE = {}


def _build():
    mm_dt = mybir.dt.bfloat16
    f16 = mybir.dt.float16
    f32 = mybir.dt.float32

    nc = bacc.Bacc()
    xt_d = nc.dram_tensor("xt", [D, S], mm_dt, kind="ExternalInput")
    wqkvt_d = nc.dram_tensor("wqkvt", [D, QCOLS + KVCOLS], mm_dt, kind="ExternalInput")
    wot_d = nc.dram_tensor("wot", [QCOLS, D], mm_dt, kind="ExternalInput")
    cos_d = nc.dram_tensor("cos64", [HD, S], f16, kind="ExternalInput")
    swap_d = nc.dram_tensor("swap64", [HD, S], f16, kind="ExternalInput")
    masks_d = nc.dram_tensor("masks", [P, P], mm_dt, kind="ExternalInput")
    ones_d = nc.dram_tensor("ones64", [1, HD], f32, kind="ExternalInput")
    ident_d = nc.dram_tensor("ident", [HD, HD], mm_dt, kind="ExternalInput")
    out_d = nc.dram_tensor("out", [S, D], f32, kind="ExternalOutput")

    with tile.TileContext(nc) as tc:
        with (
            tc.tile_pool(name="const", bufs=1) as cpool,
            tc.tile_pool(name="xt", bufs=NK) as xpool,
            tc.tile_pool(name="wq", bufs=NK) as wpool,
            tc.tile_pool(name="big", bufs=1) as bigpool,
            tc.tile_pool(name="vaug", bufs=NSQ) as vpool,
            tc.tile_pool(name="et", bufs=20) as epool,
            tc.tile_pool(name="tmp", bufs=3) as tpool,
            tc.tile_pool(name="ps_a", bufs=2, space="PSUM") as ps_a,
            tc.tile_pool(name="ps_s", bufs=2, space="PSUM") as ps_s,
            tc.tile_pool(name="ps_av", bufs=2, space="PSUM") as ps_av,
        ):
            # ---- constants / weights in ----
            # small tables via SWDGE (gpsimd); bulk via the two HWDGE
            # queues (SP + ACT) in parallel
            cos_sb = cpool.tile([HD, S], f16, tag="cos")
            swap_sb = cpool.tile([HD, S], f16, tag="swap")
            masks_sb = cpool.tile([P, P], mm_dt, tag="masks")
            ones_sb = cpool.tile([1, HD], f32, tag="ones")
            ident_sb = cpool.tile([HD, HD], mm_dt, tag="ident")
            zbias = cpool.tile([P, 1], f32, tag="zbias")
            nc.gpsimd.memset(zbias[:], 0.0)
            nc.gpsimd.dma_start(cos_sb[:], cos_d[:])
            nc.gpsimd.dma_start(swap_sb[:], swap_d[:])
            nc.gpsimd.dma_start(masks_sb[:], masks_d[:])
            nc.gpsimd.dma_start(ones_sb[:], ones_d[:])
            nc.gpsimd.dma_start(ident_sb[:], ident_d[:])

            hwdge = [nc.sync, nc.scalar]
            xt_sb = []
            wq_sb = []
            for k in range(NK):
                w = wpool.tile([P, QCOLS + KVCOLS], mm_dt, tag="wq", name=f"wqkv{k}")
                hwdge[k % 2].dma_start(w[:], wqkvt_d[k * P : (k + 1) * P, :])
                wq_sb.append(w)
            for k in range(NK):
                xt_sb.append(xpool.tile([P, S], mm_dt, tag="xt", name=f"xt{k}"))
            for q in range(NCH):
                qs = slice(q * CH, (q + 1) * CH)
                for k in range(NK):
                    eng = [nc.sync, nc.scalar, nc.gpsimd][k % 3] if q == 0 else hwdge[k % 2]
                    eng.dma_start(xt_sb[k][:, qs], xt_d[k * P : (k + 1) * P, qs])

            wot_sb = []
            for k in range(2):
                t = cpool.tile([P, D], mm_dt, tag=f"wot{k}", name=f"wot{k}")
                nc.gpsimd.dma_start(t[:], wot_d[k * P : (k + 1) * P, :])
                wot_sb.append(t)

            qt_sb = [bigpool.tile([P, S], f16, tag=f"qt{m}", name=f"qt{m}") for m in range(QH)]
            kt_sb = bigpool.tile([P, S], f16, tag="kt")
            vt_sb = bigpool.tile([HD, S], mm_dt, tag="vt")
            yt_sb = [bigpool.tile([P, S], mm_dt, tag=f"yt{m}", name=f"yt{m}") for m in range(2)]

            # ---- projections: qkvT[col, sq] accumulated over d ----
            # Mtile order: kv first so SDPA can start as soon as q is ready.
            # Mtile 2: [kT; vT] | Mtile 0: q heads 0,1 | Mtile 1: q heads 2,3
            def rope(dst, ps, chunk):
                # dst[:, chunk] = RoPE(ps) for one 64-row de-interleaved head.
                # Drain psum to f16 SBUF once so the elementwise ops run in
                # the DVE 2-byte SBUF fast mode.
                qr = tpool.tile([HD, CH], f16, tag="rope_qr", name="rope_qr")
                nc.vector.tensor_copy(qr[:], ps[:])
                t2 = tpool.tile([HD, CH], f16, tag="rope_t2", name="rope_t2")
                nc.vector.tensor_mul(t2[0:32, :], qr[32:64, :], swap_sb[32:64, chunk])
                nc.vector.tensor_mul(t2[32:64, :], qr[0:32, :], swap_sb[0:32, chunk])
                nc.vector.tensor_mul(dst[:, chunk], qr[:], cos_sb[:, chunk])
                nc.vector.tensor_add(dst[:, chunk], dst[:, chunk], t2[:])

            vaug_sb = [None] * NSQ

            def vtrans(jlist):
                with nc.named_scope("vtrans"):
                    for i in jlist:
                        pt = ps_av.tile([P, HD], mm_dt, tag="av", name="ps_vt")
                        nc.tensor.transpose(
                            pt[:], vt_sb[:, i * P : (i + 1) * P], ident_sb[:]
                        )
                        va = vpool.tile([P, HD + 1], mm_dt, tag="vaug", name=f"vaug{i}")
                        nc.vector.tensor_copy(va[:, 0:HD], pt[:])
                        nc.gpsimd.memset(va[:, HD : HD + 1], 1.0)
                        vaug_sb[i] = va

            for m in (2, 0, 1):
                if m == 1:
                    vtrans(range(NSQ))
                for j in range(NCH):
                    chunk = slice(j * CH, (j + 1) * CH)
                    ps = ps_a.tile([P, CH], f32, tag="proj", name="ps_proj")
                    with nc.named_scope("proj"):
                        for k in range(NK):
                            nc.tensor.matmul(
                                ps[:],
                                wq_sb[k][:, m * P : (m + 1) * P],
                                xt_sb[k][:, chunk],
                                start=(k == 0),
                                stop=(k == NK - 1),
                            )
                    with nc.named_scope("rope"):
                        if m < 2:
                            rope(qt_sb[2 * m][0:HD, :], ps[0:HD, :], chunk)
                            rope(qt_sb[2 * m + 1][0:HD, :], ps[HD:P, :], chunk)
                            for hh in (2 * m, 2 * m + 1):
                                nc.gpsimd.dma_start(
                                    qt_sb[hh][HD:P, chunk], qt_sb[hh][0:HD, chunk]
                                )
                        else:
                            rope(kt_sb[0:HD, :], ps[0:HD, :], chunk)
                            nc.gpsimd.dma_start(kt_sb[HD:P, chunk], kt_sb[0:HD, chunk])
                            nc.vector.tensor_copy(vt_sb[:, chunk], ps[HD:P, :])

            # ---- SDPA per (head, sq-chunk), causal ----
            # sk-tile pairs run concurrently in the PE array via row groups
            # (K=64): pair element 0 in rows 0-63, element 1 in rows 64-127.
            # Each pair writes one [128, 1024] 2-bank psum tile so the exp
            # over both halves is a single ACT op. Boundary tiles
            # (o = i-4j >= 0) only compute/exp columns [128*o:512); the
            # first 128 of those get the triangular mask.
            for j in range(NCH):
                for h in range(QH):
                    qrow = (h % 2) * HD
                    chunk = slice(j * CH, (j + 1) * CH)
                    nlive = 4 * j + 4  # sk tiles 0..4j+3 are causal-live
                    offs = [max(0, (i - 4 * j)) * P for i in range(nlive)]
                    ets = []
                    with nc.named_scope("scores"):
                        for i in range(0, nlive, 2):
                            ps2 = ps_s.tile([P, 2 * CH], f32, tag="sc", name="ps_sc")
                            for u in range(2):
                                off = offs[i + u]
                                rg = slice(u * HD, (u + 1) * HD)
                                nc.tensor.matmul(
                                    ps2[:, u * CH + off : (u + 1) * CH],
                                    kt_sb[rg, (i + u) * P : (i + u + 1) * P],
                                    qt_sb[h][rg, j * CH + off : (j + 1) * CH],
                                    start=True,
                                    stop=True,
                                )
                            et2 = epool.tile([P, 2 * CH], mm_dt, tag="et", name="et")
                            with nc.named_scope("exp"):
                                if offs[i] == 0 and offs[i + 1] == 0:
                                    nc.scalar.activation(
                                        et2[:],
                                        ps2[:],
                                        mybir.ActivationFunctionType.Exp,
                                        bias=zbias[:],
                                        scale=0.125,
                                    )
                                else:
                                    for u in range(2):
                                        off = offs[i + u]
                                        nc.scalar.activation(
                                            et2[:, u * CH + off : (u + 1) * CH],
                                            ps2[:, u * CH + off : (u + 1) * CH],
                                            mybir.ActivationFunctionType.Exp,
                                            bias=zbias[:],
                                            scale=0.125,
                                        )
                            for u in range(2):
                                if i + u >= nlive - 4:  # boundary tile
                                    off = u * CH + offs[i + u]
                                    with nc.named_scope("mask"):
                                        nc.vector.tensor_mul(
                                            et2[:, off : off + P],
                                            et2[:, off : off + P],
                                            masks_sb[:],
                                        )
                            ets.append(et2)
                    pav = ps_av.tile([HD + 1, CH], f32, tag="av", name="ps_av")
                    with nc.named_scope("av"):
                        for i in range(nlive):
                            off = offs[i]
                            nc.tensor.matmul(
                                pav[:, off:],
                                vaug_sb[i][:],
                                ets[i // 2][:, (i % 2) * CH + off : (i % 2 + 1) * CH],
                                start=(i == 0),
                                stop=(i == nlive - 1),
                            )
                    # normalize: yT = avT[0:64] / avT[64]
                    with nc.named_scope("norm"):
                        recip = tpool.tile([1, CH], f32, tag="recip", name="recip")
                        nc.vector.reciprocal(recip[:], pav[HD : HD + 1, :])
                        bc = tpool.tile([HD, CH], f32, tag="bc", name="bc")
                        nc.gpsimd.partition_broadcast(bc[:], recip[:])
                        nc.vector.tensor_mul(
                            yt_sb[h // 2][qrow : qrow + HD, chunk], pav[0:HD, :], bc[:]
                        )

            # ---- output projection partial: out[sq, d] ----
            for sm in range(NSQ):
                srow = slice(sm * P, (sm + 1) * P)
                for dcJ in range(NCH):
                    dch = slice(dcJ * CH, (dcJ + 1) * CH)
                    pw = ps_a.tile([P, CH], f32, tag="proj", name="ps_wo")
                    with nc.named_scope("wo"):
                        for k in range(2):
                            nc.tensor.matmul(
                                pw[:],
                                yt_sb[k][:, srow],
                                wot_sb[k][:, dch],
                                start=(k == 0),
                                stop=(k == 1),
                            )
                    ot = tpool.tile([P, CH], f32, tag="ot", name="ot")
                    with nc.named_scope("outdma"):
                        nc.vector.tensor_copy(ot[:], pw[:])
                        if sm >= 14:
                            half = CH // 2
                            d0 = dcJ * CH
                            nc.sync.dma_start(out_d[srow, d0 : d0 + half], ot[:, 0:half])
                            nc.gpsimd.dma_start(
                                out_d[srow, d0 + half : d0 + CH], ot[:, half:CH]
                            )
                        else:
                            odma = [nc.sync, nc.gpsimd][(sm * NCH + dcJ) % 2]
                            odma.dma_start(out_d[srow, dch], ot[:])

    nc.finalize()
    return nc


def _host_inputs(x, freqs_cos, freqs_sin, wq, wk, wv, wo):
    """Build the 8 per-core input maps (all host-side preprocessing)."""
    x = np.asarray(x, np.float32)
    cos = np.asarray(freqs_cos, np.float32)  # [S, 32]
    sin = np.asarray(freqs_sin, np.float32)
    wq = np.asarray(wq, np.float32)
    wk = np.asarray(wk, np.float32)
    wv = np.asarray(wv, np.float32)
    wo = np.asarray(wo, np.float32)

    perm = np.concatenate([np.arange(0, HD, 2), np.arange(1, HD, 2)])  # de-interleave

    xt = np.ascontiguousarray(x[0].T).astype(BF16)

    # cos64[d, t] = cos[t, d % 32]; swap64 rows 0:32 = +sin (imag-out),
    # rows 32:64 = -sin (real-out) so both tensor_mul inputs share a base
    # partition (walrus SB-SB constraint)
    cos64 = np.empty((HD, S), np.float16)
    swap64 = np.empty((HD, S), np.float16)
    for dd in range(HD):
        i = dd % 32
        cos64[dd] = cos[:, i]
        swap64[dd] = sin[:, i] if dd < 32 else -sin[:, i]

    pp = np.arange(P)[:, None]
    ff = np.arange(P)[None, :]
    masks = (pp <= ff).astype(np.float32).astype(BF16)

    ones64 = np.ones((1, HD), np.float32)
    ident = np.eye(HD, dtype=np.float32).astype(BF16)

    in_maps = []
    for c in range(NCORES):
        wq_c = wq[c * QCOLS : (c + 1) * QCOLS].reshape(QH, HD, D)[:, perm, :].reshape(
            QCOLS, D
        )
        wk_c = wk[c * HD : (c + 1) * HD][perm, :]
        wv_c = wv[c * HD : (c + 1) * HD]
        wqkvt = np.ascontiguousarray(
            np.concatenate([wq_c, wk_c, wv_c], axis=0).T
        ).astype(BF16)
        wot = np.ascontiguousarray(wo[:, c * QCOLS : (c + 1) * QCOLS].T).astype(BF16)
        in_maps.append(
            {
                "xt": xt,
                "wqkvt": wqkvt,
                "wot": wot,
                "cos64": cos64,
                "swap64": swap64,
                "masks": masks,
                "ones64": ones64,
                "ident": ident,
            }
        )
    return in_maps


def kernel(x, freqs_cos, freqs_sin, wq, wk, wv, wo):
    from concourse.bass_utils import run_bass_kernel_spmd

    if "nc" not in _CACHE:
        _CACHE["nc"] = _build()
    nc = _CACHE["nc"]
    in_maps = _host_inputs(x, freqs_cos, freqs_sin, wq, wk, wv, wo)
    res = run_bass_kernel_spmd(nc, in_maps, core_ids=list(range(NCORES)))
    out = np.zeros((S, D), np.float64)
    for r in res.results:
        out += r["out"].astype(np.float64)
    return out.astype(np.float32).reshape(1, S, D)



# revision 39
# speedup vs baseline: 1.1037x; 1.1037x over previous
"""GQA causal attention (llama3-style RoPE) on 8 TRN2 NeuronCores.

Sharding: tensor-parallel over heads. Core c gets q-heads 4c..4c+3 and
kv-head c (GQA groups intact), plus the matching row-block of wo.T.
Each core computes a full [S, D] partial of the output projection;
the host sums the 8 partials (the "all-reduce" of the row-sharded wo).

v3: fp8e4m3 + DoubleRow matmuls where the error budget allows.
  - qkv projection: 3-chain residual split (x*32 -> hi+lo, w*1024 ->
    hi+lo; chains hi@hi + lo@hi + hi@lo) in fp8-DR: exact to ~bf16 and
    25% cheaper on PE than bf16.
  - scores (chunks j>=1): q/k quantized to fp8 after RoPE, contraction
    64 = 32 partitions x 2 DR pair slots. Causal masking and DR
    dead-zone zeroing via bf16 bias matmuls (-30000) accumulated into
    the scores psum group.
  - AV (chunks j>=1): e in fp8 (exp emits fp8 directly), v residual
    (v*32 -> hi+lo) in the two DR pair slots with the et stream
    broadcast (stride-0); ones-column = 32 so the v-scale cancels in
    the softmax normalization.
  - chunk j=0 (rows 0..511) runs the SDPA in bf16: attention there is
    concentrated on few keys, so fp8 noise on scores/e would dominate
    the global max-error metric.
  - wo: bf16 (y must stay >= bf16 precision).
"""

import sys

for _p in ("/opt/trn_rl_repo", "/root/.axon_site/_ro/trn_rl_repo"):
    if _p not in sys.path:
        sys.path.insert(0, _p)

import numpy as np
import ml_dtypes

import concourse.bass as bass
from concourse.alu_op_type import AluOpType
import concourse.bacc as bacc
import concourse.mybir as mybir
import concourse.tile as tile

BF16 = ml_dtypes.bfloat16
F8 = ml_dtypes.float8_e4m3
DR = mybir.MatmulPerfMode.DoubleRow

S = 2048
D = 2048
HD = 64
NH = 32
NKV = 8
NCORES = 8
QH = NH // NCORES            # 4 local q heads
QCOLS = QH * HD              # 256
MCOLS = QCOLS + 2 * HD       # 384 (q heads | k | v)
P = 128
NKP = 8                      # contraction ktile-pairs (K = 8*256)
NSQ = S // P                 # 16 seq tiles
NCH = 4
CH = 512

SX = 32.0                    # x pre-scale into fp8 normal range
SWQ = 1024.0                 # wqkv pre-scale
SV = 32.0                    # v pre-scale (cancels via ones-col = SV)
INV_PROJ = 1.0 / (SX * SWQ)
NEG = -30000.0               # causal/dead-zone bias (exp(0.125*NEG) = 0)

_CACHE = {}


def _build():
    f32 = mybir.dt.float32
    f16 = mybir.dt.float16
    bf = mybir.dt.bfloat16
    f8 = mybir.dt.float8e4

    nc = bacc.Bacc()
    xth_d = nc.dram_tensor("xth", [NKP, P, 2, S], f8, kind="ExternalInput")
    xtl_d = nc.dram_tensor("xtl", [NKP, P, 2, S], f8, kind="ExternalInput")
    wth_d = nc.dram_tensor("wth", [NKP, P, 2, MCOLS], f8, kind="ExternalInput")
    wtl_d = nc.dram_tensor("wtl", [NKP, P, 2, MCOLS], f8, kind="ExternalInput")
    wot_d = nc.dram_tensor("wot", [QCOLS, D], bf, kind="ExternalInput")
    cos_d = nc.dram_tensor("cos128", [P, S], f16, kind="ExternalInput")
    swap_d = nc.dram_tensor("swap128", [P, S], f16, kind="ExternalInput")
    bias_d = nc.dram_tensor("bias512", [P, CH], bf, kind="ExternalInput")
    id128_d = nc.dram_tensor("id128", [P, P], bf, kind="ExternalInput")
    id64_d = nc.dram_tensor("id64", [HD, HD], f16, kind="ExternalInput")
    out_d = nc.dram_tensor("out", [S, D], bf, kind="ExternalOutput")

    with tile.TileContext(nc) as tc:
        with (
            tc.tile_pool(name="const", bufs=1) as cpool,
            tc.tile_pool(name="xin", bufs=1) as xpool,
            tc.tile_pool(name="big", bufs=1) as bigpool,
            tc.tile_pool(name="et", bufs=18) as epool,
            tc.tile_pool(name="et16", bufs=5) as e16pool,
            tc.tile_pool(name="tmp", bufs=3) as tpool,
            tc.tile_pool(name="otw", bufs=4) as opool,
            tc.tile_pool(name="ps_a", bufs=2, space="PSUM") as ps_a,
            tc.tile_pool(name="ps_s", bufs=2, space="PSUM") as ps_s,
            tc.tile_pool(name="ps_av", bufs=2, space="PSUM") as ps_av,
        ):
            # ---- constants / weights in ----
            cos_sb = cpool.tile([P, S], f16, tag="cos")
            swap_sb = cpool.tile([P, S], f16, tag="swap")
            bias_sb = cpool.tile([P, CH], bf, tag="bias")
            id128_sb = cpool.tile([P, P], bf, tag="id128")
            id64_sb = cpool.tile([HD, HD], f16, tag="id64")
            zbias = cpool.tile([P, 1], f32, tag="zbias")
            nc.gpsimd.memset(zbias[:], 0.0)
            nc.gpsimd.dma_start(cos_sb[:], cos_d[:])
            nc.gpsimd.dma_start(swap_sb[:], swap_d[:])
            nc.gpsimd.dma_start(bias_sb[:], bias_d[:])
            nc.gpsimd.dma_start(id128_sb[:], id128_d[:])
            nc.gpsimd.dma_start(id64_sb[:], id64_d[:])

            # load order tuned for startup: SP: wh, xh-half0, wl, wot;
            # ACT: xh-half1 (early idle window); Pool(SWDGE): tables + xl
            xh_sb = [xpool.tile([P, 2, S], f8, tag=f"xh{t}", name=f"xh{t}") for t in range(NKP)]
            xl_sb = [xpool.tile([P, 2, S], f8, tag=f"xl{t}", name=f"xl{t}") for t in range(NKP)]
            h0 = slice(0, S // 2)
            h1 = slice(S // 2, S)
            wh_sb, wl_sb = [], []
            c0 = slice(0, CH)
            r13 = slice(CH, S)
            for t in range(NKP):
                w = cpool.tile([P, 2, MCOLS], f8, tag=f"wh{t}", name=f"wh{t}")
                nc.sync.dma_start(w[:], wth_d[t])
                wh_sb.append(w)
            for t in range(NKP):
                nc.sync.dma_start(xh_sb[t][:, :, c0], xth_d[t][:, :, c0])
                nc.gpsimd.dma_start(xl_sb[t][:, :, c0], xtl_d[t][:, :, c0])
            for t in range(NKP):
                w = cpool.tile([P, 2, MCOLS], f8, tag=f"wl{t}", name=f"wl{t}")
                nc.sync.dma_start(w[:], wtl_d[t])
                wl_sb.append(w)
            for t in range(NKP):
                nc.scalar.dma_start(xh_sb[t][:, :, r13], xth_d[t][:, :, r13])
                nc.gpsimd.dma_start(xl_sb[t][:, :, r13], xtl_d[t][:, :, r13])
            wot_sb = []
            for k in range(2):
                t_ = cpool.tile([P, D], bf, tag=f"wot{k}", name=f"wot{k}")
                nc.sync.dma_start(t_[:], wot_d[k * P : (k + 1) * P, :])
                wot_sb.append(t_)

            # ---- per-head / kv tensors ----
            # fp8 pair-layout q/k for DR scores: [32, 2, S], pair dim =
            # head-dim halves (d = i*32 + p)
            qt8p = [bigpool.tile([32, 2, S], f8, tag=f"qt8p{h}", name=f"qt8p{h}") for h in range(QH)]
            kt8p = bigpool.tile([32, 2, S], f8, tag="kt8p")
            # rope staging [rows 0:64 = head A (re 0:32 | im 32:64), 64:128 = head B]
            qt8f = [bigpool.tile([P, S], f8, tag=f"qt8f{m}", name=f"qt8f{m}") for m in range(2)]
            kt8f = bigpool.tile([HD, S], f8, tag="kt8f")
            # f16 q/k for the bf16 j=0 path (chunk 0 columns only)
            qt16 = [bigpool.tile([HD, CH], f16, tag=f"qt16{h}", name=f"qt16{h}") for h in range(QH)]
            kt16 = bigpool.tile([HD, CH], f16, tag="kt16")
            vt_sb = bigpool.tile([HD, S], f16, tag="vt")
            yt_sb = [bigpool.tile([P, S], bf, tag=f"yt{m}", name=f"yt{m}") for m in range(2)]

            vaug = [None] * NSQ   # [128, 2, 65] fp8: (v_hi*32 | ones=32), (v_lo | 0)
            vaugb = [None] * 4    # [128, 65] bf16 for j=0

            def vtrans(jc):
                with nc.named_scope("vtrans"):
                    for i in range(4 * jc, 4 * jc + 4):
                        pt = ps_av.tile([P, HD], f16, tag="av", name="ps_vt")
                        nc.tensor.transpose(
                            pt[:], vt_sb[:, i * P : (i + 1) * P], id64_sb[:]
                        )
                        # free-dim padded to 80: DoubleRow ldweights requires
                        # pair-slot stride % 16 == 0
                        va = bigpool.tile([P, 2, HD + 16], f8, tag=f"vaug{i}", name=f"vaug{i}")
                        nc.scalar.activation(
                            va[:, 0, 0:HD], pt[:],
                            mybir.ActivationFunctionType.Copy,
                            bias=0.0, scale=SV,
                        )
                        nc.vector.scalar_tensor_tensor(
                            va[:, 1, 0:HD], pt[:], SV, va[:, 0, 0:HD],
                            op0=AluOpType.mult, op1=AluOpType.subtract,
                        )
                        nc.gpsimd.memset(va[:, 0, HD : HD + 1], SV)
                        nc.gpsimd.memset(va[:, 1, HD : HD + 1], 0.0)
                        vaug[i] = va
                        if jc == 0:
                            vb = bigpool.tile([P, HD + 1], bf, tag=f"vaugb{i}", name=f"vaugb{i}")
                            nc.scalar.copy(vb[:, 0:HD], pt[:])
                            nc.gpsimd.memset(vb[:, HD : HD + 1], 1.0)
                            vaugb[i] = vb

            # ---- qkv projection + rope ----
            # Mtile order: kv first so SDPA can start as soon as q is ready.
            # m=2: [kT(64) | vT(64)] | m=0: q heads 0,1 | m=1: q heads 2,3
            def do_proj(m, j):
                chunk = slice(j * CH, (j + 1) * CH)
                ps = ps_a.tile([P, CH], f32, tag="proj", name="ps_proj")
                msl = slice(m * P, (m + 1) * P)
                with nc.named_scope("proj"):
                    n_mm = 3 * NKP
                    i_mm = 0
                    for wset, xset in ((wh_sb, xh_sb), (wh_sb, xl_sb), (wl_sb, xh_sb)):
                        for t in range(NKP):
                            nc.tensor.matmul(
                                ps[:],
                                wset[t][:, :, msl],
                                xset[t][:, :, chunk],
                                start=(i_mm == 0),
                                stop=(i_mm == n_mm - 1),
                                perf_mode=DR,
                            )
                            i_mm += 1
                with nc.named_scope("rope"):
                    # drain psum (scaled) to f16 staging
                    nrow = P if m < 2 else HD
                    qr = tpool.tile([P, CH], f16, tag="rope_qr", name="rope_qr")
                    if m == 1:
                        nc.vector.tensor_scalar_mul(qr[:], ps[:], INV_PROJ)
                    else:
                        nc.scalar.activation(
                            qr[:], ps[:],
                            mybir.ActivationFunctionType.Copy,
                            bias=0.0, scale=INV_PROJ,
                        )
                    t2 = tpool.tile([P, CH], f16, tag="rope_t2", name="rope_t2")
                    for rb in range(nrow // HD):
                        r0 = rb * HD
                        nc.vector.tensor_mul(
                            t2[r0 : r0 + 32, :], qr[r0 + 32 : r0 + HD, :],
                            swap_sb[r0 + 32 : r0 + HD, chunk],
                        )
                        nc.vector.tensor_mul(
                            t2[r0 + 32 : r0 + HD, :], qr[r0 : r0 + 32, :],
                            swap_sb[r0 : r0 + 32, chunk],
                        )
                    t3 = tpool.tile([P, CH], f16, tag="rope_t3", name="rope_t3")
                    nc.vector.tensor_mul(t3[0:nrow, :], qr[0:nrow, :], cos_sb[0:nrow, chunk])
                    if m < 2:
                        nc.vector.tensor_add(qt8f[m][:, chunk], t3[:], t2[:])
                        if j == 0:
                            nc.vector.tensor_add(qt16[2 * m][:], t3[0:HD, :], t2[0:HD, :])
                            nc.vector.tensor_add(qt16[2 * m + 1][:], t3[HD:P, :], t2[HD:P, :])
                    else:
                        nc.vector.tensor_add(kt8f[:, chunk], t3[0:HD, :], t2[0:HD, :])
                        if j == 0:
                            nc.vector.tensor_add(kt16[:], t3[0:HD, :], t2[0:HD, :])
                        nc.vector.tensor_copy(vt_sb[:, chunk], qr[HD:P, :])
                        vtrans(j)
                # pair-fold rows (0:32 | 32:64) -> slots (0 | 1), per chunk
                with nc.named_scope("fold"):
                    if m < 2:
                        for hh in range(2):
                            h = 2 * m + hh
                            r0 = hh * HD
                            nc.sync.dma_start(qt8p[h][:, 0, chunk], qt8f[m][r0 : r0 + 32, chunk])
                            nc.sync.dma_start(qt8p[h][:, 1, chunk], qt8f[m][r0 + 32 : r0 + HD, chunk])
                    else:
                        nc.sync.dma_start(kt8p[:, 0, chunk], kt8f[0:32, chunk])
                        nc.sync.dma_start(kt8p[:, 1, chunk], kt8f[32:HD, chunk])

            # ---- SDPA (software-pipelined) ----
            # j=0: bf16 (concentrated attention -> fp8 noise too big)
            # j>=1: fp8 DR scores + fp8 e/v-residual AV
            # Pipeline: AV(j,h) is emitted after scores(j,h+1) so the PE
            # never waits on the exp of the head it just scored; wo of
            # chunk j-1 is emitted mid-way through chunk j's heads.
            def do_scores(j, h, pump_fn=None):
                nlive = 4 * j + 4
                offs = [max(0, (i - 4 * j)) * P for i in range(nlive)]
                ets = []
                with nc.named_scope("scores"):
                    for pu in range(nlive // 2):
                        if pump_fn is not None:
                            pump_fn(2)
                        a, b = 2 * pu, 2 * pu + 1
                        poff = offs[a]
                        ps2 = ps_s.tile([P, 2, CH], f32, tag="sc", name="ps_sc")
                        for u, i in ((0, a), (1, b)):
                            bnd = i >= nlive - 4  # boundary: needs bias
                            if j == 0:
                                nc.tensor.matmul(
                                    ps2[:, u, poff:CH],
                                    kt16[:, i * P : (i + 1) * P],
                                    qt16[h][:, poff:CH],
                                    start=True,
                                    stop=not bnd,
                                )
                            else:
                                nc.tensor.matmul(
                                    ps2[:, u, poff:CH],
                                    kt8p[:, :, i * P : (i + 1) * P],
                                    qt8p[h][:, :, j * CH + poff : (j + 1) * CH],
                                    start=True,
                                    stop=not bnd,
                                    perf_mode=DR,
                                )
                            if bnd:
                                dw = offs[i] - poff
                                nc.tensor.matmul(
                                    ps2[:, u, poff : offs[i] + P],
                                    id128_sb[:],
                                    bias_sb[:, 384 - dw : CH],
                                    start=False,
                                    stop=True,
                                )
                        with nc.named_scope("exp"):
                            if j == 0:
                                et = e16pool.tile([P, 2, CH], bf, tag="et16", name="et16")
                            else:
                                et = epool.tile([P, 2, CH], f8, tag="et", name="et")
                            nc.scalar.activation(
                                et[:, :, poff:CH],
                                ps2[:, :, poff:CH],
                                mybir.ActivationFunctionType.Exp,
                                bias=zbias[:],
                                scale=0.125,
                            )
                        ets.append(et)
                return ets

            # ---- filler machinery ----
            # The PE consumes scores-pairs ~2x faster than ACT can exp
            # them; since engine queues are FIFO, the PE would stall on
            # psum-buffer reuse. So AV/norm/wo work is chopped into small
            # thunks and pumped between scores pairs as PE filler.
            filler = []
            wo_q = []
            done_h = {}

            def av_thunks(j, h, ets):
                nlive = 4 * j + 4
                offs = [max(0, (i - 4 * j)) * P for i in range(nlive)]
                pav = ps_av.tile([HD + 1, CH], f32, tag="av", name="ps_av")

                def mk_av(i):
                    def emit():
                        off = offs[i]
                        W = CH - off
                        with nc.named_scope("av"):
                            if j == 0:
                                nc.tensor.matmul(
                                    pav[:, off:CH],
                                    vaugb[i][:],
                                    ets[i // 2][:, i % 2, off:CH],
                                    start=(i == 0),
                                    stop=(i == nlive - 1),
                                )
                            else:
                                nc.tensor.matmul(
                                    pav[:, off:CH],
                                    vaug[i][:, :, 0 : HD + 1],
                                    ets[i // 2][:, i % 2, off:CH].unsqueeze(1).to_broadcast([P, 2, W]),
                                    start=(i == 0),
                                    stop=(i == nlive - 1),
                                    perf_mode=DR,
                                )
                    return emit

                def norm():
                    with nc.named_scope("norm"):
                        qrow = (h % 2) * HD
                        jch = slice(j * CH, (j + 1) * CH)
                        recip = tpool.tile([1, CH], f32, tag="recip", name="recip")
                        nc.vector.reciprocal(recip[:], pav[HD : HD + 1, :])
                        bc = tpool.tile([HD, CH], f32, tag="bc", name="bc")
                        nc.gpsimd.partition_broadcast(bc[:], recip[:])
                        nc.vector.tensor_mul(
                            yt_sb[h // 2][qrow : qrow + HD, jch], pav[0:HD, :], bc[:]
                        )
                    done_h[j] = done_h.get(j, 0) + 1
                    if done_h[j] == QH:
                        for sm_ in range(4 * j, 4 * j + 4):
                            filler.extend(wo_thunks(sm_))

                return [mk_av(i) for i in range(nlive)] + [norm]

            def wo_thunks(sm):
                srow = slice(sm * P, (sm + 1) * P)
                ot = opool.tile([P, D], bf, tag="ot", name="ot")

                def mk_dc(dcJ):
                    def emit():
                        dch = slice(dcJ * CH, (dcJ + 1) * CH)
                        pw = ps_a.tile([P, CH], f32, tag="proj", name="ps_wo")
                        with nc.named_scope("wo"):
                            for k in range(2):
                                nc.tensor.matmul(
                                    pw[:],
                                    yt_sb[k][:, srow],
                                    wot_sb[k][:, dch],
                                    start=(k == 0),
                                    stop=(k == 1),
                                )
                        with nc.named_scope("outdma"):
                            if dcJ == 3:
                                nc.scalar.copy(ot[:, dch], pw[:])
                            else:
                                nc.vector.tensor_copy(ot[:, dch], pw[:])
                    return emit

                def dma():
                    with nc.named_scope("outdma"):
                        nc.sync.dma_start(out_d[srow, 0 : D // 2], ot[:, 0 : D // 2])
                        nc.gpsimd.dma_start(out_d[srow, D // 2 : D], ot[:, D // 2 : D])

                return [mk_dc(d) for d in range(NCH)] + [dma]

            def pump(n):
                done = 0
                while done < n:
                    if not filler:
                        if not wo_q:
                            return
                        filler.extend(wo_thunks(wo_q.pop(0)))
                    filler.pop(0)()
                    done += 1

            def step(j, h):
                ets = do_scores(j, h, pump)
                filler.extend(av_thunks(j, h, ets))

            emission = [
                ("p", 2, 0), ("p", 0, 0),
                ("p", 2, 1), ("p", 0, 1), ("s", 0, 0), ("s", 0, 1), ("p", 1, 0),
                ("p", 2, 2), ("p", 0, 2), ("s", 1, 0), ("s", 1, 1), ("p", 1, 1),
                ("s", 0, 2), ("s", 0, 3),
                ("p", 2, 3), ("p", 0, 3), ("s", 2, 0), ("s", 2, 1), ("p", 1, 2),
                ("s", 1, 2), ("s", 1, 3),
                ("s", 3, 0), ("s", 3, 1), ("p", 1, 3), ("s", 2, 2), ("s", 2, 3),
                ("s", 3, 2), ("s", 3, 3),
            ]
            for kind, a, b in emission:
                if kind == "p":
                    do_proj(a, b)
                else:
                    step(a, b)
            while filler or wo_q:
                pump(1000)

    nc.finalize()
    return nc


def _host_inputs(x, freqs_cos, freqs_sin, wq, wk, wv, wo):
    """Build the 8 per-core input maps (all host-side preprocessing)."""
    x = np.asarray(x, np.float32)
    cos = np.asarray(freqs_cos, np.float32)  # [S, 32]
    sin = np.asarray(freqs_sin, np.float32)
    wq = np.asarray(wq, np.float32)
    wk = np.asarray(wk, np.float32)
    wv = np.asarray(wv, np.float32)
    wo = np.asarray(wo, np.float32)

    perm = np.concatenate([np.arange(0, HD, 2), np.arange(1, HD, 2)])  # de-interleave

    xt = np.ascontiguousarray(x[0].T) * SX
    xh = xt.astype(F8)
    xl = (xt - xh.astype(np.float32)).astype(F8)

    def pairs(a, ncol):
        return np.ascontiguousarray(
            a.reshape(NKP, 2, P, ncol).transpose(0, 2, 1, 3)
        )

    xh_p = pairs(xh, S)
    xl_p = pairs(xl, S)

    # cos128[d, t] = cos[t, d % 32]; swap128 rows (d%64)<32 = +sin else -sin
    cos128 = np.empty((P, S), np.float16)
    swap128 = np.empty((P, S), np.float16)
    for dd in range(P):
        i = dd % 32
        cos128[dd] = cos[:, i]
        swap128[dd] = sin[:, i] if (dd % HD) < 32 else -sin[:, i]

    # bias512: cols 0:384 = NEG (dead zones); cols 384+c = NEG if c < p (strict upper)
    bias512 = np.full((P, CH), NEG, np.float32)
    pp = np.arange(P)[:, None]
    cc = np.arange(P)[None, :]
    bias512[:, 384:] = np.where(cc < pp, NEG, 0.0)
    bias512 = bias512.astype(BF16)

    id128 = np.eye(P, dtype=np.float32).astype(BF16)
    id64 = np.eye(HD, dtype=np.float32).astype(np.float16)

    in_maps = []
    for c in range(NCORES):
        wq_c = wq[c * QCOLS : (c + 1) * QCOLS].reshape(QH, HD, D)[:, perm, :].reshape(
            QCOLS, D
        )
        wk_c = wk[c * HD : (c + 1) * HD][perm, :]
        wv_c = wv[c * HD : (c + 1) * HD]
        wqkvt = np.ascontiguousarray(
            np.concatenate([wq_c, wk_c, wv_c], axis=0).T
        ) * SWQ
        wh = wqkvt.astype(F8)
        wl = (wqkvt - wh.astype(np.float32)).astype(F8)
        wot = np.ascontiguousarray(wo[:, c * QCOLS : (c + 1) * QCOLS].T).astype(BF16)
        in_maps.append(
            {
                "xth": xh_p,
                "xtl": xl_p,
                "wth": pairs(wh, MCOLS),
                "wtl": pairs(wl, MCOLS),
                "wot": wot,
                "cos128": cos128,
                "swap128": swap128,
                "bias512": bias512,
                "id128": id128,
                "id64": id64,
            }
        )
    return in_maps


def kernel(x, freqs_cos, freqs_sin, wq, wk, wv, wo):
    from concourse.bass_utils import run_bass_kernel_spmd

    if "nc" not in _CACHE:
        _CACHE["nc"] = _build()
    nc = _CACHE["nc"]
    in_maps = _host_inputs(x, freqs_cos, freqs_sin, wq, wk, wv, wo)
    res = run_bass_kernel_spmd(nc, in_maps, core_ids=list(range(NCORES)))
    out = np.zeros((S, D), np.float64)
    for r in res.results:
        out += r["out"].astype(np.float64)
    return out.astype(np.float32).reshape(1, S, D)


# revision 46
# speedup vs baseline: 1.1975x; 1.0850x over previous
"""GQA causal attention (llama3-style RoPE) on 8 TRN2 NeuronCores.

Sharding: tensor-parallel over heads. Core c gets q-heads 4c..4c+3 and
kv-head c (GQA groups intact), plus the matching row-block of wo.T.
Each core computes a full [S, D] partial of the output projection;
the host sums the 8 partials (the "all-reduce" of the row-sharded wo).

v3: fp8e4m3 + DoubleRow matmuls where the error budget allows.
  - qkv projection: 3-chain residual split (x*32 -> hi+lo, w*1024 ->
    hi+lo; chains hi@hi + lo@hi + hi@lo) in fp8-DR: exact to ~bf16 and
    25% cheaper on PE than bf16.
  - scores (chunks j>=1): q/k quantized to fp8 after RoPE, contraction
    64 = 32 partitions x 2 DR pair slots. Causal masking and DR
    dead-zone zeroing via bf16 bias matmuls (-30000) accumulated into
    the scores psum group.
  - AV (chunks j>=1): e in fp8 (exp emits fp8 directly), v residual
    (v*32 -> hi+lo) in the two DR pair slots with the et stream
    broadcast (stride-0); ones-column = 32 so the v-scale cancels in
    the softmax normalization.
  - chunk j=0 (rows 0..511) runs the SDPA in bf16: attention there is
    concentrated on few keys, so fp8 noise on scores/e would dominate
    the global max-error metric.
  - wo: bf16 (y must stay >= bf16 precision).
"""

import sys

for _p in ("/opt/trn_rl_repo", "/root/.axon_site/_ro/trn_rl_repo"):
    if _p not in sys.path:
        sys.path.insert(0, _p)

import numpy as np
import ml_dtypes

import concourse.bass as bass
from concourse.alu_op_type import AluOpType
import concourse.bacc as bacc
import concourse.mybir as mybir
import concourse.tile as tile

BF16 = ml_dtypes.bfloat16
F8 = ml_dtypes.float8_e4m3
DR = mybir.MatmulPerfMode.DoubleRow

S = 2048
D = 2048
HD = 64
NH = 32
NKV = 8
NCORES = 8
QH = NH // NCORES            # 4 local q heads
QCOLS = QH * HD              # 256
MCOLS = QCOLS + 2 * HD       # 384 (q heads | k | v)
P = 128
NKP = 8                      # contraction ktile-pairs (K = 8*256)
NSQ = S // P                 # 16 seq tiles
NCH = 4
CH = 512

SX = 32.0                    # x pre-scale into fp8 normal range
SWQ = 1024.0                 # wqkv pre-scale
SV = 32.0                    # v pre-scale (cancels via ones-col = SV)
INV_PROJ = 1.0 / (SX * SWQ)
NEG = -30000.0               # causal/dead-zone bias (exp(0.125*NEG) = 0)

_CACHE = {}


def _build():
    f32 = mybir.dt.float32
    f16 = mybir.dt.float16
    bf = mybir.dt.bfloat16
    f8 = mybir.dt.float8e4

    nc = bacc.Bacc()
    xth_d = nc.dram_tensor("xth", [NKP, P, 2, S], f8, kind="ExternalInput")
    xtl_d = nc.dram_tensor("xtl", [NKP, P, 2, S], f8, kind="ExternalInput")
    wth_d = nc.dram_tensor("wth", [NKP, P, 2, MCOLS], f8, kind="ExternalInput")
    wtl_d = nc.dram_tensor("wtl", [NKP, P, 2, MCOLS], f8, kind="ExternalInput")
    wot_d = nc.dram_tensor("wot", [QCOLS, D], bf, kind="ExternalInput")
    cos_d = nc.dram_tensor("cos128", [P, S], f16, kind="ExternalInput")
    swap_d = nc.dram_tensor("swap128", [P, S], f16, kind="ExternalInput")
    bias_d = nc.dram_tensor("bias512", [P, CH], bf, kind="ExternalInput")
    id128_d = nc.dram_tensor("id128", [P, P], bf, kind="ExternalInput")
    id64_d = nc.dram_tensor("id64", [HD, HD], f16, kind="ExternalInput")
    out_d = nc.dram_tensor("out", [S, D], bf, kind="ExternalOutput")

    with tile.TileContext(nc) as tc:
        with (
            tc.tile_pool(name="const", bufs=1) as cpool,
            tc.tile_pool(name="xin", bufs=1) as xpool,
            tc.tile_pool(name="big", bufs=1) as bigpool,
            tc.tile_pool(name="et", bufs=18) as epool,
            tc.tile_pool(name="et16", bufs=5) as e16pool,
            tc.tile_pool(name="tmp", bufs=3) as tpool,
            tc.tile_pool(name="otw", bufs=4) as opool,
            tc.tile_pool(name="ps_a", bufs=2, space="PSUM") as ps_a,
            tc.tile_pool(name="ps_s", bufs=2, space="PSUM") as ps_s,
            tc.tile_pool(name="ps_av", bufs=2, space="PSUM") as ps_av,
        ):
            # ---- constants / weights in ----
            cos_sb = cpool.tile([P, S], f16, tag="cos")
            swap_sb = cpool.tile([P, S], f16, tag="swap")
            bias_sb = cpool.tile([P, CH], bf, tag="bias")
            id128_sb = cpool.tile([P, P], bf, tag="id128")
            id64_sb = cpool.tile([HD, HD], f16, tag="id64")
            zbias = cpool.tile([P, 1], f32, tag="zbias")
            nc.gpsimd.memset(zbias[:], 0.0)
            _defer_tables = True
            nc.gpsimd.dma_start(bias_sb[:], bias_d[:])
            nc.gpsimd.dma_start(id128_sb[:], id128_d[:])
            nc.gpsimd.dma_start(id64_sb[:], id64_d[:])

            # load order tuned for startup: SP: wh, xh-half0, wl, wot;
            # ACT: xh-half1 (early idle window); Pool(SWDGE): tables + xl
            xh_sb = [xpool.tile([P, 2, S], f8, tag=f"xh{t}", name=f"xh{t}") for t in range(NKP)]
            xl_sb = [xpool.tile([P, 2, S], f8, tag=f"xl{t}", name=f"xl{t}") for t in range(NKP)]
            h0 = slice(0, S // 2)
            h1 = slice(S // 2, S)
            wh_sb, wl_sb = [], []
            c0 = slice(0, CH)
            r13 = slice(CH, S)
            for t in range(NKP):
                w = cpool.tile([P, 2, MCOLS], f8, tag=f"wh{t}", name=f"wh{t}")
                nc.sync.dma_start(w[:], wth_d[t])
                wh_sb.append(w)
            for t in range(NKP):
                nc.sync.dma_start(xh_sb[t][:, :, c0], xth_d[t][:, :, c0])
                nc.gpsimd.dma_start(xl_sb[t][:, :, c0], xtl_d[t][:, :, c0])
            nc.sync.dma_start(cos_sb[:], cos_d[:])
            nc.sync.dma_start(swap_sb[:], swap_d[:])
            for t in range(NKP):
                w = cpool.tile([P, 2, MCOLS], f8, tag=f"wl{t}", name=f"wl{t}")
                nc.sync.dma_start(w[:], wtl_d[t])
                wl_sb.append(w)
            for t in range(NKP):
                nc.scalar.dma_start(xh_sb[t][:, :, r13], xth_d[t][:, :, r13])
                nc.gpsimd.dma_start(xl_sb[t][:, :, r13], xtl_d[t][:, :, r13])
            wot_sb = []
            for k in range(2):
                t_ = cpool.tile([P, D], bf, tag=f"wot{k}", name=f"wot{k}")
                nc.sync.dma_start(t_[:], wot_d[k * P : (k + 1) * P, :])
                wot_sb.append(t_)

            # ---- per-head / kv tensors ----
            # fp8 pair-layout q/k for DR scores: [32, 2, S], pair dim =
            # head-dim halves (d = i*32 + p)
            qt8p = [bigpool.tile([32, 2, S], f8, tag=f"qt8p{h}", name=f"qt8p{h}") for h in range(QH)]
            kt8p = bigpool.tile([32, 2, S], f8, tag="kt8p")
            # rope staging [rows 0:64 = head A (re 0:32 | im 32:64), 64:128 = head B]
            qt8f = [bigpool.tile([P, S], f8, tag=f"qt8f{m}", name=f"qt8f{m}") for m in range(2)]
            kt8f = bigpool.tile([HD, S], f8, tag="kt8f")
            # f16 q/k for the bf16 j=0 path (chunk 0 columns only)
            qt16 = [bigpool.tile([HD, CH], f16, tag=f"qt16{h}", name=f"qt16{h}") for h in range(QH)]
            kt16 = bigpool.tile([HD, CH], f16, tag="kt16")
            vt_sb = bigpool.tile([HD, S], f16, tag="vt")
            yt_sb = [bigpool.tile([P, S], bf, tag=f"yt{m}", name=f"yt{m}") for m in range(2)]

            vaug = [None] * NSQ   # [128, 2, 65] fp8: (v_hi*32 | ones=32), (v_lo | 0)
            vaugb = [None] * 4    # [128, 65] bf16 for j=0

            def vtrans(jc):
                with nc.named_scope("vtrans"):
                    for i in range(4 * jc, 4 * jc + 4):
                        pt = ps_av.tile([P, HD], f16, tag="av", name="ps_vt")
                        nc.tensor.transpose(
                            pt[:], vt_sb[:, i * P : (i + 1) * P], id64_sb[:]
                        )
                        # free-dim padded to 80: DoubleRow ldweights requires
                        # pair-slot stride % 16 == 0
                        va = bigpool.tile([P, 2, HD + 16], f8, tag=f"vaug{i}", name=f"vaug{i}")
                        nc.vector.tensor_scalar_mul(va[:, 0, 0:HD], pt[:], SV)
                        nc.vector.scalar_tensor_tensor(
                            va[:, 1, 0:HD], pt[:], SV, va[:, 0, 0:HD],
                            op0=AluOpType.mult, op1=AluOpType.subtract,
                        )
                        nc.gpsimd.memset(va[:, 0, HD : HD + 1], SV)
                        nc.gpsimd.memset(va[:, 1, HD : HD + 1], 0.0)
                        vaug[i] = va
                        if jc == 0:
                            vb = bigpool.tile([P, HD + 1], bf, tag=f"vaugb{i}", name=f"vaugb{i}")
                            nc.scalar.copy(vb[:, 0:HD], pt[:])
                            nc.gpsimd.memset(vb[:, HD : HD + 1], 1.0)
                            vaugb[i] = vb

            # ---- qkv projection + rope ----
            # Mtile order: kv first so SDPA can start as soon as q is ready.
            # m=2: [kT(64) | vT(64)] | m=0: q heads 0,1 | m=1: q heads 2,3
            def do_proj(m, j):
                chunk = slice(j * CH, (j + 1) * CH)
                ps = ps_a.tile([P, CH], f32, tag="proj", name="ps_proj")
                msl = slice(m * P, (m + 1) * P)
                with nc.named_scope("proj"):
                    n_mm = 3 * NKP
                    i_mm = 0
                    for wset, xset in ((wh_sb, xh_sb), (wh_sb, xl_sb), (wl_sb, xh_sb)):
                        for t in range(NKP):
                            nc.tensor.matmul(
                                ps[:],
                                wset[t][:, :, msl],
                                xset[t][:, :, chunk],
                                start=(i_mm == 0),
                                stop=(i_mm == n_mm - 1),
                                perf_mode=DR,
                            )
                            i_mm += 1
                with nc.named_scope("rope"):
                    # drain psum (scaled) to f16 staging
                    nrow = P if m < 2 else HD
                    qr = tpool.tile([P, CH], f16, tag="rope_qr", name="rope_qr")
                    nc.vector.tensor_scalar_mul(qr[:], ps[:], INV_PROJ)
                    t2 = tpool.tile([P, CH], f16, tag="rope_t2", name="rope_t2")
                    for rb in range(nrow // HD):
                        r0 = rb * HD
                        nc.gpsimd.tensor_mul(
                            t2[r0 : r0 + 32, :], qr[r0 + 32 : r0 + HD, :],
                            swap_sb[r0 + 32 : r0 + HD, chunk],
                        )
                        nc.gpsimd.tensor_mul(
                            t2[r0 + 32 : r0 + HD, :], qr[r0 : r0 + 32, :],
                            swap_sb[r0 : r0 + 32, chunk],
                        )
                    t3 = tpool.tile([P, CH], f16, tag="rope_t3", name="rope_t3")
                    nc.vector.tensor_mul(t3[0:nrow, :], qr[0:nrow, :], cos_sb[0:nrow, chunk])
                    if m < 2:
                        nc.vector.tensor_add(qt8f[m][:, chunk], t3[:], t2[:])
                        if j == 0:
                            nc.vector.tensor_add(qt16[2 * m][:], t3[0:HD, :], t2[0:HD, :])
                            nc.vector.tensor_add(qt16[2 * m + 1][:], t3[HD:P, :], t2[HD:P, :])
                    else:
                        nc.vector.tensor_add(kt8f[:, chunk], t3[0:HD, :], t2[0:HD, :])
                        if j == 0:
                            nc.vector.tensor_add(kt16[:], t3[0:HD, :], t2[0:HD, :])
                        nc.vector.tensor_copy(vt_sb[:, chunk], qr[HD:P, :])
                        vtrans(j)
                # pair-fold rows (0:32 | 32:64) -> slots (0 | 1), per chunk
                with nc.named_scope("fold"):
                    if m < 2:
                        for hh in range(2):
                            h = 2 * m + hh
                            r0 = hh * HD
                            nc.sync.dma_start(qt8p[h][:, 0, chunk], qt8f[m][r0 : r0 + 32, chunk])
                            nc.sync.dma_start(qt8p[h][:, 1, chunk], qt8f[m][r0 + 32 : r0 + HD, chunk])
                    else:
                        nc.sync.dma_start(kt8p[:, 0, chunk], kt8f[0:32, chunk])
                        nc.sync.dma_start(kt8p[:, 1, chunk], kt8f[32:HD, chunk])

            # ---- SDPA (software-pipelined) ----
            # j=0: bf16 (concentrated attention -> fp8 noise too big)
            # j>=1: fp8 DR scores + fp8 e/v-residual AV
            # Pipeline: AV(j,h) is emitted after scores(j,h+1) so the PE
            # never waits on the exp of the head it just scored; wo of
            # chunk j-1 is emitted mid-way through chunk j's heads.
            def do_scores(j, h, pump_fn=None):
                nlive = 4 * j + 4
                offs = [max(0, (i - 4 * j)) * P for i in range(nlive)]
                ets = []
                with nc.named_scope("scores"):
                    for pu in range(nlive // 2):
                        if pump_fn is not None:
                            pump_fn(2)
                        a, b = 2 * pu, 2 * pu + 1
                        poff = offs[a]
                        ps2 = ps_s.tile([P, 2, CH], f32, tag="sc", name="ps_sc")
                        for u, i in ((0, a), (1, b)):
                            bnd = i >= nlive - 4  # boundary: needs bias
                            if j == 0:
                                nc.tensor.matmul(
                                    ps2[:, u, poff:CH],
                                    kt16[:, i * P : (i + 1) * P],
                                    qt16[h][:, poff:CH],
                                    start=True,
                                    stop=not bnd,
                                )
                            else:
                                nc.tensor.matmul(
                                    ps2[:, u, poff:CH],
                                    kt8p[:, :, i * P : (i + 1) * P],
                                    qt8p[h][:, :, j * CH + poff : (j + 1) * CH],
                                    start=True,
                                    stop=not bnd,
                                    perf_mode=DR,
                                )
                            if bnd:
                                dw = offs[i] - poff
                                nc.tensor.matmul(
                                    ps2[:, u, poff : offs[i] + P],
                                    id128_sb[:],
                                    bias_sb[:, 384 - dw : CH],
                                    start=False,
                                    stop=True,
                                )
                        with nc.named_scope("exp"):
                            if j == 0:
                                et = e16pool.tile([P, 2, CH], bf, tag="et16", name="et16")
                            else:
                                et = epool.tile([P, 2, CH], f8, tag="et", name="et")
                            nc.scalar.activation(
                                et[:, :, poff:CH],
                                ps2[:, :, poff:CH],
                                mybir.ActivationFunctionType.Exp,
                                bias=zbias[:],
                                scale=0.125,
                            )
                        ets.append(et)
                return ets

            # ---- filler machinery ----
            # The PE consumes scores-pairs ~2x faster than ACT can exp
            # them; since engine queues are FIFO, the PE would stall on
            # psum-buffer reuse. So AV/norm/wo work is chopped into small
            # thunks and pumped between scores pairs as PE filler.
            filler = []
            wo_q = []
            done_h = {}

            def av_thunks(j, h, ets):
                nlive = 4 * j + 4
                offs = [max(0, (i - 4 * j)) * P for i in range(nlive)]
                pav = ps_av.tile([HD + 1, CH], f32, tag="av", name="ps_av")

                def mk_av(i):
                    def emit():
                        off = offs[i]
                        W = CH - off
                        with nc.named_scope("av"):
                            if j == 0:
                                nc.tensor.matmul(
                                    pav[:, off:CH],
                                    vaugb[i][:],
                                    ets[i // 2][:, i % 2, off:CH],
                                    start=(i == 0),
                                    stop=(i == nlive - 1),
                                )
                            else:
                                nc.tensor.matmul(
                                    pav[:, off:CH],
                                    vaug[i][:, :, 0 : HD + 1],
                                    ets[i // 2][:, i % 2, off:CH].unsqueeze(1).to_broadcast([P, 2, W]),
                                    start=(i == 0),
                                    stop=(i == nlive - 1),
                                    perf_mode=DR,
                                )
                    return emit

                def norm():
                    with nc.named_scope("norm"):
                        qrow = (h % 2) * HD
                        jch = slice(j * CH, (j + 1) * CH)
                        recip = tpool.tile([1, CH], f32, tag="recip", name="recip")
                        nc.vector.reciprocal(recip[:], pav[HD : HD + 1, :])
                        bc = tpool.tile([HD, CH], f32, tag="bc", name="bc")
                        nc.gpsimd.partition_broadcast(bc[:], recip[:])
                        nc.vector.tensor_mul(
                            yt_sb[h // 2][qrow : qrow + HD, jch], pav[0:HD, :], bc[:]
                        )
                    done_h[j] = done_h.get(j, 0) + 1
                    if done_h[j] == QH:
                        for sm_ in range(4 * j, 4 * j + 4):
                            filler.extend(wo_thunks(sm_))

                return [mk_av(i) for i in range(nlive)] + [norm]

            def wo_thunks(sm):
                srow = slice(sm * P, (sm + 1) * P)
                ot = opool.tile([P, D], bf, tag="ot", name="ot")

                def mk_dc(dcJ):
                    def emit():
                        dch = slice(dcJ * CH, (dcJ + 1) * CH)
                        pw = ps_a.tile([P, CH], f32, tag="proj", name="ps_wo")
                        with nc.named_scope("wo"):
                            for k in range(2):
                                nc.tensor.matmul(
                                    pw[:],
                                    yt_sb[k][:, srow],
                                    wot_sb[k][:, dch],
                                    start=(k == 0),
                                    stop=(k == 1),
                                )
                        with nc.named_scope("outdma"):
                            if sm >= 12:
                                nc.scalar.copy(ot[:, dch], pw[:])
                            else:
                                nc.vector.tensor_copy(ot[:, dch], pw[:])
                    return emit

                def dma():
                    with nc.named_scope("outdma"):
                        nc.sync.dma_start(out_d[srow, 0 : D // 2], ot[:, 0 : D // 2])
                        nc.gpsimd.dma_start(out_d[srow, D // 2 : D], ot[:, D // 2 : D])

                return [mk_dc(d) for d in range(NCH)] + [dma]

            def pump(n):
                done = 0
                while done < n:
                    if not filler:
                        if not wo_q:
                            return
                        filler.extend(wo_thunks(wo_q.pop(0)))
                    filler.pop(0)()
                    done += 1

            def step(j, h):
                ets = do_scores(j, h, pump)
                filler.extend(av_thunks(j, h, ets))

            emission = [
                ("p", 2, 0), ("p", 0, 0),
                ("p", 2, 1), ("p", 0, 1), ("s", 0, 0), ("s", 0, 1), ("p", 1, 0),
                ("p", 2, 2), ("p", 0, 2), ("s", 1, 0), ("s", 1, 1), ("p", 1, 1),
                ("s", 0, 2), ("s", 0, 3),
                ("p", 2, 3), ("p", 0, 3), ("s", 2, 0), ("s", 2, 1), ("p", 1, 2),
                ("s", 1, 2), ("s", 1, 3),
                ("s", 3, 0), ("s", 3, 1), ("p", 1, 3), ("s", 2, 2), ("s", 2, 3),
                ("s", 3, 2), ("s", 3, 3),
            ]
            for kind, a, b in emission:
                if kind == "p":
                    do_proj(a, b)
                else:
                    step(a, b)
            while filler or wo_q:
                pump(1000)

    nc.finalize()
    return nc


def _host_inputs(x, freqs_cos, freqs_sin, wq, wk, wv, wo):
    """Build the 8 per-core input maps (all host-side preprocessing)."""
    x = np.asarray(x, np.float32)
    cos = np.asarray(freqs_cos, np.float32)  # [S, 32]
    sin = np.asarray(freqs_sin, np.float32)
    wq = np.asarray(wq, np.float32)
    wk = np.asarray(wk, np.float32)
    wv = np.asarray(wv, np.float32)
    wo = np.asarray(wo, np.float32)

    perm = np.concatenate([np.arange(0, HD, 2), np.arange(1, HD, 2)])  # de-interleave

    xt = np.ascontiguousarray(x[0].T) * SX
    xh = xt.astype(F8)
    xl = (xt - xh.astype(np.float32)).astype(F8)

    def pairs(a, ncol):
        return np.ascontiguousarray(
            a.reshape(NKP, 2, P, ncol).transpose(0, 2, 1, 3)
        )

    xh_p = pairs(xh, S)
    xl_p = pairs(xl, S)

    # cos128[d, t] = cos[t, d % 32]; swap128 rows (d%64)<32 = +sin else -sin
    cos128 = np.empty((P, S), np.float16)
    swap128 = np.empty((P, S), np.float16)
    for dd in range(P):
        i = dd % 32
        cos128[dd] = cos[:, i]
        swap128[dd] = sin[:, i] if (dd % HD) < 32 else -sin[:, i]

    # bias512: cols 0:384 = NEG (dead zones); cols 384+c = NEG if c < p (strict upper)
    bias512 = np.full((P, CH), NEG, np.float32)
    pp = np.arange(P)[:, None]
    cc = np.arange(P)[None, :]
    bias512[:, 384:] = np.where(cc < pp, NEG, 0.0)
    bias512 = bias512.astype(BF16)

    id128 = np.eye(P, dtype=np.float32).astype(BF16)
    id64 = np.eye(HD, dtype=np.float32).astype(np.float16)

    in_maps = []
    for c in range(NCORES):
        wq_c = wq[c * QCOLS : (c + 1) * QCOLS].reshape(QH, HD, D)[:, perm, :].reshape(
            QCOLS, D
        )
        wk_c = wk[c * HD : (c + 1) * HD][perm, :]
        wv_c = wv[c * HD : (c + 1) * HD]
        wqkvt = np.ascontiguousarray(
            np.concatenate([wq_c, wk_c, wv_c], axis=0).T
        ) * SWQ
        wh = wqkvt.astype(F8)
        wl = (wqkvt - wh.astype(np.float32)).astype(F8)
        wot = np.ascontiguousarray(wo[:, c * QCOLS : (c + 1) * QCOLS].T).astype(BF16)
        in_maps.append(
            {
                "xth": xh_p,
                "xtl": xl_p,
                "wth": pairs(wh, MCOLS),
                "wtl": pairs(wl, MCOLS),
                "wot": wot,
                "cos128": cos128,
                "swap128": swap128,
                "bias512": bias512,
                "id128": id128,
                "id64": id64,
            }
        )
    return in_maps


def kernel(x, freqs_cos, freqs_sin, wq, wk, wv, wo):
    from concourse.bass_utils import run_bass_kernel_spmd

    if "nc" not in _CACHE:
        _CACHE["nc"] = _build()
    nc = _CACHE["nc"]
    in_maps = _host_inputs(x, freqs_cos, freqs_sin, wq, wk, wv, wo)
    res = run_bass_kernel_spmd(nc, in_maps, core_ids=list(range(NCORES)))
    out = np.zeros((S, D), np.float64)
    for r in res.results:
        out += r["out"].astype(np.float64)
    return out.astype(np.float32).reshape(1, S, D)


# revision 51
# speedup vs baseline: 1.2053x; 1.0065x over previous
"""GQA causal attention (llama3-style RoPE) on 8 TRN2 NeuronCores.

Sharding: tensor-parallel over heads. Core c gets q-heads 4c..4c+3 and
kv-head c (GQA groups intact), plus the matching row-block of wo.T.
Each core computes a full [S, D] partial of the output projection;
the host sums the 8 partials (the "all-reduce" of the row-sharded wo).

v3: fp8e4m3 + DoubleRow matmuls where the error budget allows.
  - qkv projection: 3-chain residual split (x*32 -> hi+lo, w*1024 ->
    hi+lo; chains hi@hi + lo@hi + hi@lo) in fp8-DR: exact to ~bf16 and
    25% cheaper on PE than bf16.
  - scores (chunks j>=1): q/k quantized to fp8 after RoPE, contraction
    64 = 32 partitions x 2 DR pair slots. Causal masking and DR
    dead-zone zeroing via bf16 bias matmuls (-30000) accumulated into
    the scores psum group.
  - AV (chunks j>=1): e in fp8 (exp emits fp8 directly), v residual
    (v*32 -> hi+lo) in the two DR pair slots with the et stream
    broadcast (stride-0); ones-column = 32 so the v-scale cancels in
    the softmax normalization.
  - chunk j=0 (rows 0..511) runs the SDPA in bf16: attention there is
    concentrated on few keys, so fp8 noise on scores/e would dominate
    the global max-error metric.
  - wo: bf16 (y must stay >= bf16 precision).
"""

import sys

for _p in ("/opt/trn_rl_repo", "/root/.axon_site/_ro/trn_rl_repo"):
    if _p not in sys.path:
        sys.path.insert(0, _p)

import numpy as np
import ml_dtypes

import concourse.bass as bass
from concourse.alu_op_type import AluOpType
import concourse.bacc as bacc
import concourse.mybir as mybir
import concourse.tile as tile

BF16 = ml_dtypes.bfloat16
F8 = ml_dtypes.float8_e4m3
DR = mybir.MatmulPerfMode.DoubleRow

S = 2048
D = 2048
HD = 64
NH = 32
NKV = 8
NCORES = 8
QH = NH // NCORES            # 4 local q heads
QCOLS = QH * HD              # 256
MCOLS = QCOLS + 2 * HD       # 384 (q heads | k | v)
P = 128
NKP = 8                      # contraction ktile-pairs (K = 8*256)
NSQ = S // P                 # 16 seq tiles
NCH = 4
CH = 512

SX = 32.0                    # x pre-scale into fp8 normal range
SWQ = 1024.0                 # wqkv pre-scale
SV = 32.0                    # v pre-scale (cancels via ones-col = SV)
INV_PROJ = 1.0 / (SX * SWQ)
NEG = -30000.0               # causal/dead-zone bias (exp(0.125*NEG) = 0)

_CACHE = {}


def _build():
    f32 = mybir.dt.float32
    f16 = mybir.dt.float16
    bf = mybir.dt.bfloat16
    f8 = mybir.dt.float8e4

    nc = bacc.Bacc()
    xth_d = nc.dram_tensor("xth", [NKP, P, 2, S], f8, kind="ExternalInput")
    xtl_d = nc.dram_tensor("xtl", [NKP, P, 2, S], f8, kind="ExternalInput")
    wth_d = nc.dram_tensor("wth", [NKP, P, 2, MCOLS], f8, kind="ExternalInput")
    wtl_d = nc.dram_tensor("wtl", [NKP, P, 2, MCOLS], f8, kind="ExternalInput")
    wot_d = nc.dram_tensor("wot", [QCOLS, D], bf, kind="ExternalInput")
    cos_d = nc.dram_tensor("cos128", [P, S], f16, kind="ExternalInput")
    swap_d = nc.dram_tensor("swap128", [P, S], f16, kind="ExternalInput")
    bias_d = nc.dram_tensor("bias512", [P, CH], bf, kind="ExternalInput")
    masks8_d = nc.dram_tensor("masks8", [P, P], f8, kind="ExternalInput")
    id128_d = nc.dram_tensor("id128", [P, P], bf, kind="ExternalInput")
    id64_d = nc.dram_tensor("id64", [HD, HD], f16, kind="ExternalInput")
    out_d = nc.dram_tensor("out", [S, D], bf, kind="ExternalOutput")

    with tile.TileContext(nc) as tc:
        with (
            tc.tile_pool(name="const", bufs=1) as cpool,
            tc.tile_pool(name="xin", bufs=1) as xpool,
            tc.tile_pool(name="big", bufs=1) as bigpool,
            tc.tile_pool(name="et", bufs=18) as epool,
            tc.tile_pool(name="et16", bufs=5) as e16pool,
            tc.tile_pool(name="tmp", bufs=3) as tpool,
            tc.tile_pool(name="otw", bufs=4) as opool,
            tc.tile_pool(name="ps_a", bufs=2, space="PSUM") as ps_a,
            tc.tile_pool(name="ps_s", bufs=2, space="PSUM") as ps_s,
            tc.tile_pool(name="ps_av", bufs=2, space="PSUM") as ps_av,
        ):
            # ---- constants / weights in ----
            cos_sb = cpool.tile([P, S], f16, tag="cos")
            swap_sb = cpool.tile([P, S], f16, tag="swap")
            bias_sb = cpool.tile([P, CH], bf, tag="bias")
            masks8_sb = cpool.tile([P, P], f8, tag="masks8")
            id128_sb = cpool.tile([P, P], bf, tag="id128")
            id64_sb = cpool.tile([HD, HD], f16, tag="id64")
            zbias = cpool.tile([P, 1], f32, tag="zbias")
            nc.gpsimd.memset(zbias[:], 0.0)
            _defer_tables = True
            nc.gpsimd.dma_start(bias_sb[:], bias_d[:])
            nc.gpsimd.dma_start(masks8_sb[:], masks8_d[:])
            nc.gpsimd.dma_start(id128_sb[:], id128_d[:])
            nc.gpsimd.dma_start(id64_sb[:], id64_d[:])

            # load order tuned for startup: SP: wh, xh-half0, wl, wot;
            # ACT: xh-half1 (early idle window); Pool(SWDGE): tables + xl
            xh_sb = [xpool.tile([P, 2, S], f8, tag=f"xh{t}", name=f"xh{t}") for t in range(NKP)]
            xl_sb = [xpool.tile([P, 2, S], f8, tag=f"xl{t}", name=f"xl{t}") for t in range(NKP)]
            h0 = slice(0, S // 2)
            h1 = slice(S // 2, S)
            wh_sb, wl_sb = [], []
            c0 = slice(0, CH)
            r13 = slice(CH, S)
            for t in range(NKP):
                w = cpool.tile([P, 2, MCOLS], f8, tag=f"wh{t}", name=f"wh{t}")
                nc.sync.dma_start(w[:], wth_d[t])
                wh_sb.append(w)
            for t in range(NKP):
                nc.sync.dma_start(xh_sb[t][:, :, c0], xth_d[t][:, :, c0])
                nc.gpsimd.dma_start(xl_sb[t][:, :, c0], xtl_d[t][:, :, c0])
            nc.sync.dma_start(cos_sb[:], cos_d[:])
            nc.sync.dma_start(swap_sb[:], swap_d[:])
            for t in range(NKP):
                w = cpool.tile([P, 2, MCOLS], f8, tag=f"wl{t}", name=f"wl{t}")
                nc.sync.dma_start(w[:], wtl_d[t])
                wl_sb.append(w)
            c1 = slice(CH, 2 * CH)
            c23 = slice(2 * CH, S)
            for t in range(NKP):
                nc.scalar.dma_start(xh_sb[t][:, :, c1], xth_d[t][:, :, c1])
                nc.gpsimd.dma_start(xl_sb[t][:, :, c1], xtl_d[t][:, :, c1])
            for t in range(NKP):
                nc.scalar.dma_start(xh_sb[t][:, :, c23], xth_d[t][:, :, c23])
                nc.gpsimd.dma_start(xl_sb[t][:, :, c23], xtl_d[t][:, :, c23])
            wot_sb = []
            for k in range(2):
                t_ = cpool.tile([P, D], bf, tag=f"wot{k}", name=f"wot{k}")
                nc.sync.dma_start(t_[:], wot_d[k * P : (k + 1) * P, :])
                wot_sb.append(t_)

            # ---- per-head / kv tensors ----
            # fp8 pair-layout q/k for DR scores: [32, 2, S], pair dim =
            # head-dim halves (d = i*32 + p)
            qt8p = [bigpool.tile([32, 2, S], f8, tag=f"qt8p{h}", name=f"qt8p{h}") for h in range(QH)]
            kt8p = bigpool.tile([32, 2, S], f8, tag="kt8p")
            # rope staging [rows 0:64 = head A (re 0:32 | im 32:64), 64:128 = head B]
            qt8f = [bigpool.tile([P, S], f8, tag=f"qt8f{m}", name=f"qt8f{m}") for m in range(2)]
            kt8f = bigpool.tile([HD, S], f8, tag="kt8f")
            # f16 q/k for the bf16 j=0 path (chunk 0 columns only)
            qt16 = [bigpool.tile([HD, CH], f16, tag=f"qt16{h}", name=f"qt16{h}") for h in range(QH)]
            kt16 = bigpool.tile([HD, CH], f16, tag="kt16")
            vt_sb = bigpool.tile([HD, S], f16, tag="vt")
            yt_sb = [bigpool.tile([P, S], bf, tag=f"yt{m}", name=f"yt{m}") for m in range(2)]

            vaug = [None] * NSQ   # [128, 2, 65] fp8: (v_hi*32 | ones=32), (v_lo | 0)
            vaugb = [None] * 4    # [128, 65] bf16 for j=0

            def vtrans(jc):
                with nc.named_scope("vtrans"):
                    for i in range(4 * jc, 4 * jc + 4):
                        pt = ps_av.tile([P, HD], f16, tag="av", name="ps_vt")
                        nc.tensor.transpose(
                            pt[:], vt_sb[:, i * P : (i + 1) * P], id64_sb[:]
                        )
                        # free-dim padded to 80: DoubleRow ldweights requires
                        # pair-slot stride % 16 == 0
                        va = bigpool.tile([P, 2, HD + 16], f8, tag=f"vaug{i}", name=f"vaug{i}")
                        nc.vector.tensor_scalar_mul(va[:, 0, 0:HD], pt[:], SV)
                        nc.vector.scalar_tensor_tensor(
                            va[:, 1, 0:HD], pt[:], SV, va[:, 0, 0:HD],
                            op0=AluOpType.mult, op1=AluOpType.subtract,
                        )
                        nc.gpsimd.memset(va[:, 0, HD : HD + 1], SV)
                        nc.gpsimd.memset(va[:, 1, HD : HD + 1], 0.0)
                        vaug[i] = va
                        if jc == 0:
                            vb = bigpool.tile([P, HD + 1], bf, tag=f"vaugb{i}", name=f"vaugb{i}")
                            nc.scalar.copy(vb[:, 0:HD], pt[:])
                            nc.gpsimd.memset(vb[:, HD : HD + 1], 1.0)
                            vaugb[i] = vb

            # ---- qkv projection + rope ----
            # Mtile order: kv first so SDPA can start as soon as q is ready.
            # m=2: [kT(64) | vT(64)] | m=0: q heads 0,1 | m=1: q heads 2,3
            def do_proj(m, j):
                chunk = slice(j * CH, (j + 1) * CH)
                ps = ps_a.tile([P, CH], f32, tag="proj", name="ps_proj")
                msl = slice(m * P, (m + 1) * P)
                with nc.named_scope("proj"):
                    n_mm = 3 * NKP
                    i_mm = 0
                    for wset, xset in ((wh_sb, xh_sb), (wh_sb, xl_sb), (wl_sb, xh_sb)):
                        for t in range(NKP):
                            nc.tensor.matmul(
                                ps[:],
                                wset[t][:, :, msl],
                                xset[t][:, :, chunk],
                                start=(i_mm == 0),
                                stop=(i_mm == n_mm - 1),
                                perf_mode=DR,
                            )
                            i_mm += 1
                with nc.named_scope("rope"):
                    # drain psum (scaled) to f16 staging
                    nrow = P if m < 2 else HD
                    qr = tpool.tile([P, CH], f16, tag="rope_qr", name="rope_qr")
                    nc.vector.tensor_scalar_mul(qr[:], ps[:], INV_PROJ)
                    t2 = tpool.tile([P, CH], f16, tag="rope_t2", name="rope_t2")
                    for rb in range(nrow // HD):
                        r0 = rb * HD
                        nc.gpsimd.tensor_mul(
                            t2[r0 : r0 + 32, :], qr[r0 + 32 : r0 + HD, :],
                            swap_sb[r0 + 32 : r0 + HD, chunk],
                        )
                        nc.gpsimd.tensor_mul(
                            t2[r0 + 32 : r0 + HD, :], qr[r0 : r0 + 32, :],
                            swap_sb[r0 : r0 + 32, chunk],
                        )
                    t3 = tpool.tile([P, CH], f16, tag="rope_t3", name="rope_t3")
                    nc.vector.tensor_mul(t3[0:nrow, :], qr[0:nrow, :], cos_sb[0:nrow, chunk])
                    if m < 2:
                        nc.vector.tensor_add(qt8f[m][:, chunk], t3[:], t2[:])
                        if j == 0:
                            nc.vector.tensor_add(qt16[2 * m][:], t3[0:HD, :], t2[0:HD, :])
                            nc.vector.tensor_add(qt16[2 * m + 1][:], t3[HD:P, :], t2[HD:P, :])
                    else:
                        nc.vector.tensor_add(kt8f[:, chunk], t3[0:HD, :], t2[0:HD, :])
                        if j == 0:
                            nc.vector.tensor_add(kt16[:], t3[0:HD, :], t2[0:HD, :])
                        nc.vector.tensor_copy(vt_sb[:, chunk], qr[HD:P, :])
                        vtrans(j)
                # pair-fold rows (0:32 | 32:64) -> slots (0 | 1), per chunk
                with nc.named_scope("fold"):
                    if m < 2:
                        for hh in range(2):
                            h = 2 * m + hh
                            r0 = hh * HD
                            nc.sync.dma_start(qt8p[h][:, 0, chunk], qt8f[m][r0 : r0 + 32, chunk])
                            nc.sync.dma_start(qt8p[h][:, 1, chunk], qt8f[m][r0 + 32 : r0 + HD, chunk])
                    else:
                        nc.sync.dma_start(kt8p[:, 0, chunk], kt8f[0:32, chunk])
                        nc.sync.dma_start(kt8p[:, 1, chunk], kt8f[32:HD, chunk])

            # ---- SDPA (software-pipelined) ----
            # j=0: bf16 (concentrated attention -> fp8 noise too big)
            # j>=1: fp8 DR scores + fp8 e/v-residual AV
            # Pipeline: AV(j,h) is emitted after scores(j,h+1) so the PE
            # never waits on the exp of the head it just scored; wo of
            # chunk j-1 is emitted mid-way through chunk j's heads.
            def do_scores(j, h, pump_fn=None):
                nlive = 4 * j + 4
                offs = [max(0, (i - 4 * j)) * P for i in range(nlive)]
                ets = []
                with nc.named_scope("scores"):
                    for pu in range(nlive // 2):
                        if pump_fn is not None:
                            pump_fn(2)
                        a, b = 2 * pu, 2 * pu + 1
                        poff = offs[a]
                        ps2 = ps_s.tile([P, 2, CH], f32, tag="sc", name="ps_sc")
                        for u, i in ((0, a), (1, b)):
                            bnd = i >= nlive - 4  # boundary: needs bias
                            if j == 0:
                                nc.tensor.matmul(
                                    ps2[:, u, poff:CH],
                                    kt16[:, i * P : (i + 1) * P],
                                    qt16[h][:, poff:CH],
                                    start=True,
                                    stop=not bnd,
                                )
                            else:
                                nc.tensor.matmul(
                                    ps2[:, u, poff:CH],
                                    kt8p[:, :, i * P : (i + 1) * P],
                                    qt8p[h][:, :, j * CH + poff : (j + 1) * CH],
                                    start=True,
                                    stop=not (bnd and (offs[i] - poff) > 0),
                                    perf_mode=DR,
                                )
                            if bnd:
                                dw = offs[i] - poff
                                if j == 0 or dw > 0:
                                    nc.tensor.matmul(
                                        ps2[:, u, poff : (offs[i] + P if j == 0 else offs[i])],
                                        id128_sb[:],
                                        bias_sb[:, 384 - dw : (CH if j == 0 else 384)],
                                        start=False,
                                        stop=True,
                                    )
                        with nc.named_scope("exp"):
                            if j == 0:
                                et = e16pool.tile([P, 2, CH], bf, tag="et16", name="et16")
                            else:
                                et = epool.tile([P, 2, CH], f8, tag="et", name="et")
                            nc.scalar.activation(
                                et[:, :, poff:CH],
                                ps2[:, :, poff:CH],
                                mybir.ActivationFunctionType.Exp,
                                bias=zbias[:],
                                scale=0.125,
                            )
                        if j > 0:
                            for u, i in ((0, a), (1, b)):
                                if i >= nlive - 4:
                                    off = offs[i]
                                    with nc.named_scope("mask"):
                                        nc.vector.tensor_mul(
                                            et[:, u, off : off + P],
                                            et[:, u, off : off + P],
                                            masks8_sb[:],
                                        )
                        ets.append(et)
                return ets

            # ---- filler machinery ----
            # The PE consumes scores-pairs ~2x faster than ACT can exp
            # them; since engine queues are FIFO, the PE would stall on
            # psum-buffer reuse. So AV/norm/wo work is chopped into small
            # thunks and pumped between scores pairs as PE filler.
            filler = []
            wo_q = []
            done_h = {}

            def av_thunks(j, h, ets):
                nlive = 4 * j + 4
                offs = [max(0, (i - 4 * j)) * P for i in range(nlive)]
                pav = ps_av.tile([HD + 1, CH], f32, tag="av", name="ps_av")

                def mk_av(i):
                    def emit():
                        off = offs[i]
                        W = CH - off
                        with nc.named_scope("av"):
                            if j == 0:
                                nc.tensor.matmul(
                                    pav[:, off:CH],
                                    vaugb[i][:],
                                    ets[i // 2][:, i % 2, off:CH],
                                    start=(i == 0),
                                    stop=(i == nlive - 1),
                                )
                            else:
                                nc.tensor.matmul(
                                    pav[:, off:CH],
                                    vaug[i][:, :, 0 : HD + 1],
                                    ets[i // 2][:, i % 2, off:CH].unsqueeze(1).to_broadcast([P, 2, W]),
                                    start=(i == 0),
                                    stop=(i == nlive - 1),
                                    perf_mode=DR,
                                )
                    return emit

                def norm():
                    with nc.named_scope("norm"):
                        qrow = (h % 2) * HD
                        jch = slice(j * CH, (j + 1) * CH)
                        recip = tpool.tile([1, CH], f32, tag="recip", name="recip")
                        nc.vector.reciprocal(recip[:], pav[HD : HD + 1, :])
                        bc = tpool.tile([HD, CH], f32, tag="bc", name="bc")
                        nc.gpsimd.partition_broadcast(bc[:], recip[:])
                        nc.vector.tensor_mul(
                            yt_sb[h // 2][qrow : qrow + HD, jch], pav[0:HD, :], bc[:]
                        )
                    done_h[j] = done_h.get(j, 0) + 1
                    if done_h[j] == QH:
                        for sm_ in range(4 * j, 4 * j + 4):
                            filler.extend(wo_thunks(sm_))

                return [mk_av(i) for i in range(nlive)] + [norm]

            def wo_thunks(sm):
                srow = slice(sm * P, (sm + 1) * P)
                ot = opool.tile([P, D], bf, tag="ot", name="ot")

                def mk_dc(dcJ):
                    def emit():
                        dch = slice(dcJ * CH, (dcJ + 1) * CH)
                        pw = ps_a.tile([P, CH], f32, tag="proj", name="ps_wo")
                        with nc.named_scope("wo"):
                            for k in range(2):
                                nc.tensor.matmul(
                                    pw[:],
                                    yt_sb[k][:, srow],
                                    wot_sb[k][:, dch],
                                    start=(k == 0),
                                    stop=(k == 1),
                                )
                        with nc.named_scope("outdma"):
                            if sm >= 12 and dcJ % 2 == 0:
                                nc.scalar.copy(ot[:, dch], pw[:])
                            else:
                                nc.vector.tensor_copy(ot[:, dch], pw[:])
                    return emit

                def dma():
                    with nc.named_scope("outdma"):
                        nc.sync.dma_start(out_d[srow, 0 : D // 2], ot[:, 0 : D // 2])
                        nc.gpsimd.dma_start(out_d[srow, D // 2 : D], ot[:, D // 2 : D])

                return [mk_dc(d) for d in range(NCH)] + [dma]

            def pump(n):
                done = 0
                while done < n:
                    if not filler:
                        if not wo_q:
                            return
                        filler.extend(wo_thunks(wo_q.pop(0)))
                    filler.pop(0)()
                    done += 1

            def step(j, h):
                ets = do_scores(j, h, pump)
                filler.extend(av_thunks(j, h, ets))

            emission = [
                ("p", 2, 0), ("p", 0, 0),
                ("p", 2, 1), ("p", 0, 1), ("s", 0, 0), ("s", 0, 1), ("p", 1, 0),
                ("p", 2, 2), ("p", 0, 2), ("s", 1, 0), ("s", 1, 1), ("p", 1, 1),
                ("s", 0, 2), ("s", 0, 3),
                ("p", 2, 3), ("p", 0, 3), ("s", 2, 0), ("s", 2, 1), ("p", 1, 2),
                ("s", 1, 2), ("s", 1, 3),
                ("s", 3, 0), ("s", 3, 1), ("p", 1, 3), ("s", 2, 2), ("s", 2, 3),
                ("s", 3, 2), ("s", 3, 3),
            ]
            for kind, a, b in emission:
                if kind == "p":
                    do_proj(a, b)
                else:
                    step(a, b)
            while filler or wo_q:
                pump(1000)

    nc.finalize()
    return nc


def _host_inputs(x, freqs_cos, freqs_sin, wq, wk, wv, wo):
    """Build the 8 per-core input maps (all host-side preprocessing)."""
    x = np.asarray(x, np.float32)
    cos = np.asarray(freqs_cos, np.float32)  # [S, 32]
    sin = np.asarray(freqs_sin, np.float32)
    wq = np.asarray(wq, np.float32)
    wk = np.asarray(wk, np.float32)
    wv = np.asarray(wv, np.float32)
    wo = np.asarray(wo, np.float32)

    perm = np.concatenate([np.arange(0, HD, 2), np.arange(1, HD, 2)])  # de-interleave

    xt = np.ascontiguousarray(x[0].T) * SX
    xh = xt.astype(F8)
    xl = (xt - xh.astype(np.float32)).astype(F8)

    def pairs(a, ncol):
        return np.ascontiguousarray(
            a.reshape(NKP, 2, P, ncol).transpose(0, 2, 1, 3)
        )

    xh_p = pairs(xh, S)
    xl_p = pairs(xl, S)

    # cos128[d, t] = cos[t, d % 32]; swap128 rows (d%64)<32 = +sin else -sin
    cos128 = np.empty((P, S), np.float16)
    swap128 = np.empty((P, S), np.float16)
    for dd in range(P):
        i = dd % 32
        cos128[dd] = cos[:, i]
        swap128[dd] = sin[:, i] if (dd % HD) < 32 else -sin[:, i]

    # bias512: cols 0:384 = NEG (dead zones); cols 384+c = NEG if c < p (strict upper)
    bias512 = np.full((P, CH), NEG, np.float32)
    pp = np.arange(P)[:, None]
    cc = np.arange(P)[None, :]
    bias512[:, 384:] = np.where(cc < pp, NEG, 0.0)
    bias512 = bias512.astype(BF16)

    id128 = np.eye(P, dtype=np.float32).astype(BF16)
    masks8 = (np.arange(P)[:, None] <= np.arange(P)[None, :]).astype(np.float32).astype(F8)
    id64 = np.eye(HD, dtype=np.float32).astype(np.float16)

    in_maps = []
    for c in range(NCORES):
        wq_c = wq[c * QCOLS : (c + 1) * QCOLS].reshape(QH, HD, D)[:, perm, :].reshape(
            QCOLS, D
        )
        wk_c = wk[c * HD : (c + 1) * HD][perm, :]
        wv_c = wv[c * HD : (c + 1) * HD]
        wqkvt = np.ascontiguousarray(
            np.concatenate([wq_c, wk_c, wv_c], axis=0).T
        ) * SWQ
        wh = wqkvt.astype(F8)
        wl = (wqkvt - wh.astype(np.float32)).astype(F8)
        wot = np.ascontiguousarray(wo[:, c * QCOLS : (c + 1) * QCOLS].T).astype(BF16)
        in_maps.append(
            {
                "xth": xh_p,
                "xtl": xl_p,
                "wth": pairs(wh, MCOLS),
                "wtl": pairs(wl, MCOLS),
                "wot": wot,
                "cos128": cos128,
                "swap128": swap128,
                "bias512": bias512,
                "masks8": masks8,
                "id128": id128,
                "id64": id64,
            }
        )
    return in_maps


def kernel(x, freqs_cos, freqs_sin, wq, wk, wv, wo):
    from concourse.bass_utils import run_bass_kernel_spmd

    if "nc" not in _CACHE:
        _CACHE["nc"] = _build()
    nc = _CACHE["nc"]
    in_maps = _host_inputs(x, freqs_cos, freqs_sin, wq, wk, wv, wo)
    res = run_bass_kernel_spmd(nc, in_maps, core_ids=list(range(NCORES)))
    out = np.zeros((S, D), np.float64)
    for r in res.results:
        out += r["out"].astype(np.float64)
    return out.astype(np.float32).reshape(1, S, D)
